# revision 23
# baseline (speedup 1.0000x reference)
"""Bass/Trainium2 kernel for HCFC-GNN (3-layer GCN + hierarchy max-constraint).

Strategy (8 NeuronCores, SPMD, target-sharded):
  - Nodes sharded 6250/core (padded chunks of 6272). Edges (no self-loops)
    sharded by TARGET core, sorted by (target block, source half).
  - Tables carry dinv_r-scaled rows; the target-side dinv_c scale is deferred
    through ReLU (positive homogeneity) into the NEXT table's row scale
    (dinv^2 per-partition ACT scale + sqrt(deg)-row bias matmul).
  - Aggregation is TRANSPOSED: accT[f, c] = sum_e msg[e, f] * S[e, c] via
    matmul(lhsT=msg, rhs=S) so the result is already feature-major for the
    next dense transform -- no PE transposes / PSUM copies.
  - Self-loops: one identity matmul per block (lhsT = cached own table block).
  - S one-hot built with packed-innermost is_equal layout (DVE 2x mode).
  - L1 table (dinv*[x,1]) precomputed on host (no collective). L2 table
    allgathered bf16 128-wide. L3 table allgathered compact 16-wide then
    pad-expanded for the 256B-row gather constraint.
"""

import os
import numpy as np
import ml_dtypes

N = 50000
E = 1600000
C = 13
DIN = 12
H = 128
NCORES = 8
SH = N // NCORES          # 6250 nodes per shard
CH = 6272                 # padded chunk rows (6250 + 22 zero pad) = 49*128
BLK = CH // 128           # 49 blocks per shard
LASTB = SH - (BLK - 1) * 128  # 106 valid rows in last block
HALF = 4 * CH             # 25088 rows per gather half (int16-safe)
ZROW = 0                  # pad gather row: any finite row (S row is zero)
PADCREL = 300.0           # colrel value guaranteed not to match 0..127

bf16 = ml_dtypes.bfloat16

LAST_RESULTS = None


def _prep_edges(edge_index):
    """Partition/sort edges (NO self-loops); build per-core gather-index and
    colrel streams with slot sizes (TL) uniform across cores."""
    row = np.asarray(edge_index[0], np.int64)
    col = np.asarray(edge_index[1], np.int64)
    deg = (np.bincount(row, minlength=N) + 1).astype(np.float32)  # + self loop

    s_shard = row // SH
    grow = s_shard * CH + (row % SH)       # row index in gathered table
    half = (grow >= HALF).astype(np.int64)
    gloc = np.where(half == 0, grow, grow - HALF)
    tcore = col // SH
    tcol = col % SH
    blk = tcol // 128
    crel = (tcol % 128).astype(np.int64)

    key = ((tcore * BLK) + blk) * 2 + half
    order = np.lexsort((gloc, key))
    key_s = key[order]
    gloc_s = gloc[order]
    crel_s = crel[order]

    nslots = NCORES * BLK * 2
    cnt = np.bincount(key_s, minlength=nslots).reshape(NCORES, BLK, 2)
    starts = np.zeros(nslots + 1, np.int64)
    np.cumsum(cnt.reshape(-1), out=starts[1:])

    TL = np.maximum(1, ((cnt + 127) // 128).max(axis=0))  # [BLK, 2]
    off = np.zeros((BLK, 2), np.int64)
    tot = [0, 0]
    for h in (0, 1):
        for b in range(BLK):
            off[b, h] = tot[h]
            tot[h] += TL[b, h]

    gidx = []   # per core: ((gi_lo, cr_lo), (gi_hi, cr_hi))
    for k in range(NCORES):
        per_half = []
        for h in (0, 1):
            gparts, cparts = [], []
            for b in range(BLK):
                s = starts[(k * BLK + b) * 2 + h]
                e = starts[(k * BLK + b) * 2 + h + 1]
                n = int(e - s)
                m = int(TL[b, h]) * 128
                gseg = np.full(m, ZROW, np.int64)
                cseg = np.full(m, PADCREL, np.float64)
                gseg[:n] = gloc_s[s:e]
                cseg[:n] = crel_s[s:e]
                # wrapped idx layout: pos i -> partition i%16, col i//16
                gparts.append(gseg.reshape(m // 16, 16).T.astype(np.int16))
                # colrel layout: pos i -> partition i%128, col i//128
                cparts.append(cseg.reshape(m // 128, 128).T.astype(bf16))
            g = np.hstack(gparts)   # [16, 8*TOT]; replicated to 128 on device
            per_half.append((np.ascontiguousarray(g), np.hstack(cparts).copy()))
        gidx.append(per_half)
    return deg, TL, off, gidx


def _build_program(TL, off):
    import concourse.bacc as bacc
    import concourse.mybir as mybir
    import concourse.tile as tile

    dt = mybir.dt
    nc = bacc.Bacc("TRN2", target_bir_lowering=False, debug=False,
                   num_devices=NCORES)

    TOT = [int(off[-1, 0] + TL[-1, 0]), int(off[-1, 1] + TL[-1, 1])]
    TLMAX = int(TL.max())

    # ---- inputs ----
    t1pad = nc.dram_tensor("t1pad", [NCORES * CH, H], dt.bfloat16,
                           kind="ExternalInput")
    t1own = nc.dram_tensor("t1own", [CH, 16], dt.bfloat16, kind="ExternalInput")
    W1aug = nc.dram_tensor("W1aug", [16, H], dt.bfloat16, kind="ExternalInput")
    W2T = nc.dram_tensor("W2T", [H, H], dt.bfloat16, kind="ExternalInput")
    b2r = nc.dram_tensor("b2r", [1, H], dt.bfloat16, kind="ExternalInput")
    W3T = nc.dram_tensor("W3T", [H, 16], dt.bfloat16, kind="ExternalInput")
    b3r = nc.dram_tensor("b3r", [1, 16], dt.bfloat16, kind="ExternalInput")
    sqdeg = nc.dram_tensor("sqdeg", [1, CH], dt.bfloat16, kind="ExternalInput")
    dinvc = nc.dram_tensor("dinvc", [128, BLK], dt.float32, kind="ExternalInput")
    dinv2c = nc.dram_tensor("dinv2c", [128, BLK], dt.float32, kind="ExternalInput")
    id128 = nc.dram_tensor("id128", [128, H], dt.bfloat16, kind="ExternalInput")
    id16 = nc.dram_tensor("id16", [16, 16], dt.bfloat16, kind="ExternalInput")
    io3 = nc.dram_tensor("io3", [128, 128 * TLMAX], dt.bfloat16,
                         kind="ExternalInput")
    Rbf = nc.dram_tensor("Rbf", [128, C * C], dt.bfloat16, kind="ExternalInput")
    gi_lo = nc.dram_tensor("gi_lo", [16, 8 * TOT[0]], dt.int16,
                           kind="ExternalInput")
    gi_hi = nc.dram_tensor("gi_hi", [16, 8 * TOT[1]], dt.int16,
                           kind="ExternalInput")
    cr_lo = nc.dram_tensor("cr_lo", [128, TOT[0]], dt.bfloat16,
                           kind="ExternalInput")
    cr_hi = nc.dram_tensor("cr_hi", [128, TOT[1]], dt.bfloat16,
                           kind="ExternalInput")
    out = nc.dram_tensor("out", [SH, C], dt.float32, kind="ExternalOutput")

    gin2 = nc.dram_tensor("gin2", [CH, H], dt.bfloat16)
    gout2 = nc.dram_tensor("gout2", [NCORES * CH, H], dt.bfloat16,
                           addr_space="Shared")
    gin3 = nc.dram_tensor("gin3", [CH, 16], dt.bfloat16)
    gout3 = nc.dram_tensor("gout3", [NCORES * CH, 16], dt.bfloat16,
                           addr_space="Shared")
    g3pad = nc.dram_tensor("g3pad", [NCORES * CH, H], dt.bfloat16)

    with tile.TileContext(nc) as tc:
        with (
            tc.tile_pool(name="const", bufs=1) as cpool,
            tc.tile_pool(name="idx", bufs=1) as ipool,
            tc.tile_pool(name="msg", bufs=6) as mpool,
            tc.tile_pool(name="sbl", bufs=6) as spool,
            tc.tile_pool(name="hsm", bufs=4) as hpool,
            tc.tile_pool(name="psum", bufs=3, space="PSUM") as pp,
            tc.tile_pool(name="psumt", bufs=4, space="PSUM") as ppt,
        ):
            # ---- constants ----
            def load(pool, shape, dtype, src, tag):
                t = pool.tile(shape, dtype, tag=tag)
                nc.sync.dma_start(out=t[:], in_=src)
                return t

            # load idx rows into partitions 0:16, then replicate x8 on DVE
            # (dma_gather needs the wrapped idx physically in all 128 parts)
            gil_t = ipool.tile([128, 8 * TOT[0]], dt.int16, tag="gil")
            nc.sync.dma_start(out=gil_t[0:16, :], in_=gi_lo[:])
            gih_t = ipool.tile([128, 8 * TOT[1]], dt.int16, tag="gih")
            nc.sync.dma_start(out=gih_t[0:16, :], in_=gi_hi[:])
            for t in (gil_t, gih_t):
                # DVE writes need 32-partition alignment; first doubling via
                # a small SBUF->SBUF DMA instead
                nc.sync.dma_start(out=t[16:32, :], in_=t[0:16, :])
                for lo in (32, 64):
                    nc.vector.tensor_copy(t[lo:2 * lo, :], t[0:lo, :])
            crl_t = load(ipool, [128, TOT[0]], dt.bfloat16, cr_lo[:], "crl")
            crh_t = load(ipool, [128, TOT[1]], dt.bfloat16, cr_hi[:], "crh")
            w1_t = load(cpool, [16, H], dt.bfloat16, W1aug[:], "w1")
            w2_t = load(cpool, [H, H], dt.bfloat16, W2T[:], "w2")
            b2_t = load(cpool, [1, H], dt.bfloat16, b2r[:], "b2")
            w3_t = load(cpool, [H, 16], dt.bfloat16, W3T[:], "w3")
            b3_t = load(cpool, [1, 16], dt.bfloat16, b3r[:], "b3")
            sq_t = load(cpool, [1, CH], dt.bfloat16, sqdeg[:], "sq")
            dinv_t = load(cpool, [128, BLK], dt.float32, dinvc[:], "dinv")
            dinv2_t = load(cpool, [128, BLK], dt.float32, dinv2c[:], "dinv2")
            id128_t = load(cpool, [128, H], dt.bfloat16, id128[:], "id128")
            id16_t = load(cpool, [16, 16], dt.bfloat16, id16[:], "id16")
            io3_t = load(cpool, [128, 128, TLMAX], dt.bfloat16,
                         io3[:].rearrange("p (c j) -> p c j", c=128), "io3")
            r_t = load(cpool, [128, C * C], dt.bfloat16, Rbf[:], "r")
            t1o_t = load(cpool, [128, BLK, 16], dt.bfloat16,
                         t1own[:].rearrange("(b p) f -> p b f", p=128), "t1o")
            # table-block cache (own shard) for next layer's self-loop matmul
            gc2 = cpool.tile([128, BLK * 128], dt.bfloat16, tag="gc2")
            gc3 = cpool.tile([128, BLK * 16], dt.bfloat16, tag="gc3")

            halves2 = ((gil_t, crl_t), (gih_t, crh_t))

            def agg_block(b, src_lo, src_hi, width, self_lhsT, tag):
                """Transposed gather+scatter for node block b.
                Returns PSUM tile [width, 128] = sum_e msg[e,:width]^T S[e,:]
                (+ self-loop term)."""
                acc = pp.tile([width, 128], dt.float32, tag="agg")
                nc.tensor.matmul(acc[:, :], self_lhsT, id128_t[:, :],
                                 start=True, stop=False)
                srcs = (src_lo, src_hi)
                for h in (0, 1):
                    gi_t, cr_t = halves2[h]
                    tl = int(TL[b, h])
                    o = int(off[b, h])
                    msg = mpool.tile([128, TLMAX, H], dt.bfloat16, tag="msg")
                    nc.gpsimd.dma_gather(
                        out_ap=msg[:, 0:tl, :], in_ap=srcs[h],
                        idxs_ap=gi_t[:, o * 8:(o + tl) * 8],
                        num_idxs=tl * 128, num_idxs_reg=tl * 128, elem_size=H,
                        single_packet=False,
                    )
                    S = spool.tile([128, 128, TLMAX], dt.bfloat16, tag="sb")
                    nc.vector.tensor_tensor(
                        out=S[:, :, 0:tl],
                        in0=cr_t[:, o:o + tl].unsqueeze(1)
                            .broadcast_to([128, 128, tl]),
                        in1=io3_t[:, :, 0:tl],
                        op=mybir.AluOpType.is_equal,
                    )
                    for j in range(tl):
                        last = (h == 1 and j == int(TL[b, 1]) - 1)
                        nc.tensor.matmul(acc[:, :], msg[:, j, 0:width],
                                         S[:, :, j], start=False, stop=last)
                return acc

            # ---------------- Layer 1: agg(t1) -> transform -> table2 -------
            for b in range(BLK):
                bs, be = b * 128, (b + 1) * 128
                acc1 = agg_block(b, t1pad[0:HALF, :], t1pad[HALF:2 * HALF, :],
                                 16, t1o_t[:, b, :], "agg")
                htbA = hpool.tile([16, 128], dt.bfloat16, tag="htbA")
                nc.scalar.activation(htbA[:, :], acc1[:, :],
                                     mybir.ActivationFunctionType.Copy)
                acc2T = ppt.tile([H, 128], dt.float32, tag="tf")
                nc.tensor.matmul(acc2T[:, :], w1_t[:, :], htbA[:, :],
                                 start=True, stop=True)
                h1T = hpool.tile([H, 128], dt.bfloat16, tag="hT")
                nc.scalar.activation(h1T[:, :], acc2T[:, :],
                                     mybir.ActivationFunctionType.Relu)
                acc3 = ppt.tile([128, H], dt.float32, tag="tf")
                nc.tensor.matmul(acc3[:, :], h1T[:, :], w2_t[:, :],
                                 start=True, stop=False)
                nc.tensor.matmul(acc3[:, :], sq_t[0:1, bs:be], b2_t[:, :],
                                 start=False, stop=True)
                nc.scalar.activation(gc2[:, bs:be], acc3[:, :],
                                     mybir.ActivationFunctionType.Copy,
                                     scale=dinv2_t[:, b:b + 1])
                nc.sync.dma_start(out=gin2[bs:be, :], in_=gc2[:, bs:be])

            nc.gpsimd.collective_compute(
                "AllGather", mybir.AluOpType.bypass,
                replica_groups=[list(range(NCORES))],
                ins=[gin2[:, :]], outs=[gout2[:, :]],
            )

            # ---------------- Layer 2: agg(t2) -> transform -> table3 -------
            for b in range(BLK):
                bs, be = b * 128, (b + 1) * 128
                acc4 = agg_block(b, gout2[0:HALF, :], gout2[HALF:2 * HALF, :],
                                 H, gc2[:, bs:be], "agg")
                h2T = hpool.tile([H, 128], dt.bfloat16, tag="hT")
                nc.scalar.activation(h2T[:, :], acc4[:, :],
                                     mybir.ActivationFunctionType.Relu)
                acc5 = ppt.tile([128, 16], dt.float32, tag="tf")
                nc.tensor.matmul(acc5[:, :], h2T[:, :], w3_t[:, :],
                                 start=True, stop=False)
                nc.tensor.matmul(acc5[:, :], sq_t[0:1, bs:be], b3_t[:, :],
                                 start=False, stop=True)
                nc.scalar.activation(gc3[:, b * 16:(b + 1) * 16], acc5[:, :],
                                     mybir.ActivationFunctionType.Copy,
                                     scale=dinv2_t[:, b:b + 1])
                nc.sync.dma_start(out=gin3[bs:be, :],
                                  in_=gc3[:, b * 16:(b + 1) * 16])

            nc.gpsimd.collective_compute(
                "AllGather", mybir.AluOpType.bypass,
                replica_groups=[list(range(NCORES))],
                ins=[gin3[:, :]], outs=[gout3[:, :]],
            )
            # pad-expand compact 16-wide table into 256B-stride gather rows
            # (split per gather-half so half-0 gathers start during half-1)
            nc.sync.dma_start(out=g3pad[0:HALF, 0:16], in_=gout3[0:HALF, :])
            nc.sync.dma_start(out=g3pad[HALF:2 * HALF, 0:16],
                              in_=gout3[HALF:2 * HALF, :])

            # ---------------- Layer 3: agg(t3) -> sigmoid -> hierarchy max --
            for b in range(BLK):
                bs = b * 128
                acc6 = agg_block(b, g3pad[0:HALF, :], g3pad[HALF:2 * HALF, :],
                                 16, gc3[:, b * 16:(b + 1) * 16], "agg")
                sb6 = hpool.tile([16, 128], dt.bfloat16, tag="htbA")
                nc.scalar.activation(sb6[:, :], acc6[:, :],
                                     mybir.ActivationFunctionType.Copy)
                acc7 = ppt.tile([128, 16], dt.float32, tag="tf")
                nc.tensor.matmul(acc7[:, :], sb6[:, :], id16_t[:, :],
                                 start=True, stop=True)
                s7 = hpool.tile([128, 16], dt.bfloat16, tag="s7")
                nc.scalar.activation(s7[:, :], acc7[:, :],
                                     mybir.ActivationFunctionType.Sigmoid,
                                     scale=dinv_t[:, b:b + 1])
                tmp = hpool.tile([128, C, C], dt.bfloat16, tag="tmp")
                nc.vector.tensor_tensor(
                    out=tmp[:, :, :],
                    in0=s7[:, 0:C].unsqueeze(1).broadcast_to([128, C, C]),
                    in1=r_t[:, :].rearrange("p (a b) -> p a b", a=C),
                    op=mybir.AluOpType.mult,
                )
                o13 = hpool.tile([128, C], dt.float32, tag="o13")
                nc.vector.tensor_reduce(o13[:, :], tmp[:, :, :],
                                        axis=mybir.AxisListType.X,
                                        op=mybir.AluOpType.max)
                rows = 128 if b < BLK - 1 else LASTB
                nc.sync.dma_start(out=out[bs:bs + rows, :],
                                  in_=o13[0:rows, :])

    nc.compile()
    return nc


def _host_tensors(x, R, W1, b1, W2, b2, W3, b3, deg, TL):
    """Replicated + per-core host-prepared input tensors."""
    TLMAX = int(TL.max())
    dinv = 1.0 / np.sqrt(deg)

    # layer-1 table rows: dinv_r * [x_r, 1, 0...] (bf16, CH-strided chunks)
    t1pad = np.zeros([NCORES * CH, H], bf16)
    t1c = np.zeros([NCORES, CH, 16], bf16)
    for k in range(NCORES):
        sl = slice(k * SH, (k + 1) * SH)
        rows = np.zeros([CH, 16], np.float32)
        rows[:SH, :DIN] = x[sl] * dinv[sl, None]
        rows[:SH, DIN] = dinv[sl]
        t1c[k] = rows.astype(bf16)
        t1pad[k * CH:(k + 1) * CH, :16] = rows.astype(bf16)

    W1a = np.zeros([16, H], np.float32)
    W1a[:DIN] = np.asarray(W1, np.float32).T
    W1a[DIN] = np.asarray(b1, np.float32)
    W3Tp = np.zeros([H, 16], np.float32)
    W3Tp[:, :C] = np.asarray(W3, np.float32).T
    b3p = np.zeros([1, 16], np.float32)
    b3p[0, :C] = np.asarray(b3, np.float32)

    io3 = np.tile(np.arange(128, dtype=np.float32)[None, :, None],
                  (128, 1, TLMAX)).reshape(128, 128 * TLMAX)

    common = {
        "t1pad": t1pad,
        "W1aug": W1a.astype(bf16),
        "W2T": np.ascontiguousarray(np.asarray(W2, np.float32).T).astype(bf16),
        "b2r": np.asarray(b2, np.float32)[None, :].astype(bf16),
        "W3T": W3Tp.astype(bf16),
        "b3r": b3p.astype(bf16),
        "id128": np.eye(128, dtype=np.float32).astype(bf16),
        "id16": np.eye(16, dtype=np.float32).astype(bf16),
        "io3": io3.astype(bf16),
        "Rbf": np.tile(np.asarray(R, np.float32).reshape(1, C * C),
                       (128, 1)).astype(bf16),
    }

    per_core = []
    for k in range(NCORES):
        sl = slice(k * SH, (k + 1) * SH)
        sq = np.zeros([1, CH], np.float32)
        sq[0, :SH] = np.sqrt(deg[sl])
        dv = np.ones([CH], np.float32)
        dv[:SH] = dinv[sl]
        dvc = np.ascontiguousarray(dv.reshape(BLK, 128).T)
        dv2 = np.zeros([CH], np.float32)
        dv2[:SH] = dinv[sl] ** 2
        dv2c = np.ascontiguousarray(dv2.reshape(BLK, 128).T)
        per_core.append({
            "t1own": np.ascontiguousarray(t1c[k]),
            "sqdeg": sq.astype(bf16),
            "dinvc": dvc,
            "dinv2c": dv2c,
        })
    return common, per_core


def kernel(x, edge_index, R, W1, b1, W2, b2, W3, b3, **_):
    global LAST_RESULTS
    import concourse.mybir  # noqa: F401
    from concourse.bass_utils import run_bass_kernel_spmd

    x = np.asarray(x, np.float32)
    edge_index = np.asarray(edge_index, np.int32)
    deg, TL, off, gidx = _prep_edges(edge_index)

    nc = _build_program(TL, off)
    common, per_core = _host_tensors(x, R, W1, b1, W2, b2, W3, b3, deg, TL)

    in_maps = []
    for k in range(NCORES):
        (g_lo, c_lo), (g_hi, c_hi) = gidx[k]
        m = dict(common)
        m.update(per_core[k])
        m.update({"gi_lo": g_lo, "gi_hi": g_hi, "cr_lo": c_lo, "cr_hi": c_hi})
        in_maps.append(m)

    trace = os.environ.get("GNN_TRACE") == "1"
    res = run_bass_kernel_spmd(nc, in_maps, core_ids=list(range(NCORES)),
                               trace=trace)
    LAST_RESULTS = res

    reps = int(os.environ.get("GNN_BENCH", "0"))
    if reps > 0:
        _bench(nc, in_maps, reps)
    return np.concatenate([res.results[k]["out"] for k in range(NCORES)], axis=0)


BENCH_TIMES = None
BENCH_PIPELINED_NS = None


def _bench(nc, in_maps, reps):
    """Time repeated executions of the already-built program through a single
    jit instance (NEFF compile amortized away; inputs device_put once)."""
    global BENCH_TIMES
    import time
    import jax
    from jax.sharding import Mesh, PartitionSpec, NamedSharding
    from jax.experimental.shard_map import shard_map
    import concourse.mybir as mybir
    from concourse.bass2jax import (_bass_exec_p, partition_id_tensor,
                                    install_neuronx_cc_hook)

    install_neuronx_cc_hook()
    in_names, out_names, out_avals, zero_outs = [], [], [], []
    pname = nc.partition_id_tensor.name if nc.partition_id_tensor else None
    for alloc in nc.m.functions[0].allocations:
        if not isinstance(alloc, mybir.MemoryLocationSet):
            continue
        name = alloc.memorylocations[0].name
        if alloc.kind == "ExternalInput":
            if name != pname:
                in_names.append(name)
        elif alloc.kind == "ExternalOutput":
            out_names.append(name)
            shape = tuple(alloc.tensor_shape)
            dtype = mybir.dt.np(alloc.dtype)
            out_avals.append(jax.core.ShapedArray(shape, dtype))
            zero_outs.append(np.zeros(shape, dtype))
    n_params = len(in_names)
    all_names = in_names + out_names + ([pname] if pname else [])

    def _body(*args):
        ops = list(args)
        if pname:
            ops.append(partition_id_tensor())
        return tuple(_bass_exec_p.bind(
            *ops, out_avals=tuple(out_avals), in_names=tuple(all_names),
            out_names=tuple(out_names), lowering_input_output_aliases=(),
            sim_require_finite=True, sim_require_nnan=True, nc=nc))

    devices = jax.devices()[:NCORES]
    mesh = Mesh(np.asarray(devices), ("core",))
    nouts = len(out_names)
    sharded = jax.jit(
        shard_map(_body, mesh=mesh,
                  in_specs=(PartitionSpec("core"),) * (n_params + nouts),
                  out_specs=(PartitionSpec("core"),) * nouts, check_rep=False),
        donate_argnums=tuple(range(n_params, n_params + nouts)),
        keep_unused=True)
    sh = NamedSharding(mesh, PartitionSpec("core"))
    dev_in = [jax.device_put(
        np.concatenate([np.asarray(in_maps[c][nm]) for c in range(NCORES)], axis=0), sh)
        for nm in in_names]
    times = []
    for i in range(reps + 1):
        zs = [jax.device_put(
            np.zeros((NCORES * z.shape[0], *z.shape[1:]), z.dtype), sh)
            for z in zero_outs]
        t0 = time.perf_counter()
        outs = sharded(*dev_in, *zs)
        jax.block_until_ready(outs)
        times.append(time.perf_counter() - t0)
    BENCH_TIMES = times
    print("bench wall times (s):", " ".join(f"{t:.4f}" for t in times))
    print(f"bench min/median after warmup: {min(times[1:]):.4f} / "
          f"{sorted(times[1:])[len(times[1:]) // 2]:.4f}")

    # pipelined async dispatch: amortizes per-call RPC overhead; min of 3
    # rounds (wall noise through axon is ~+-1.3 ms)
    NPIPE = 20
    best = None
    for rnd in range(3):
        zss = [[jax.device_put(
            np.zeros((NCORES * z.shape[0], *z.shape[1:]), z.dtype), sh)
            for z in zero_outs] for _ in range(NPIPE)]
        t0 = time.perf_counter()
        outs = None
        for i in range(NPIPE):
            outs = sharded(*dev_in, *zss[i])
        jax.block_until_ready(outs)
        tp = (time.perf_counter() - t0) / NPIPE
        print(f"bench pipelined round {rnd}: {tp * 1e3:.3f} ms")
        best = tp if best is None else min(best, tp)
    global BENCH_PIPELINED_NS
    BENCH_PIPELINED_NS = int(best * 1e9)
    print(f"bench pipelined per-exec: {best * 1e3:.3f} ms "
          f"({best * 1e9:.0f} ns upper bound)")


# revision 28
# speedup vs baseline: 1.1317x; 1.1317x over previous
"""Bass/Trainium2 kernel for HCFC-GNN (3-layer GCN + hierarchy max-constraint).

Strategy (8 NeuronCores, SPMD, target-sharded):
  - Nodes sharded 6250/core (padded chunks of 6272). Edges (no self-loops)
    sharded by TARGET core, sorted by (target block, source half).
  - Tables carry dinv_r-scaled rows; the target-side dinv_c scale is deferred
    through ReLU (positive homogeneity) into the NEXT table's row scale
    (dinv^2 per-partition ACT scale + sqrt(deg)-row bias matmul).
  - Aggregation is TRANSPOSED: accT[f, c] = sum_e msg[e, f] * S[e, c] via
    matmul(lhsT=msg, rhs=S) so the result is already feature-major for the
    next dense transform -- no PE transposes / PSUM copies.
  - Self-loops: one identity matmul per block (lhsT = cached own table block).
  - S one-hot built with packed-innermost is_equal layout (DVE 2x mode).
  - L1 table (dinv*[x,1]) precomputed on host (no collective). L2 table
    allgathered bf16 128-wide. L3 table allgathered compact 16-wide then
    pad-expanded for the 256B-row gather constraint.
"""

import os
import numpy as np
import ml_dtypes

N = 50000
E = 1600000
C = 13
DIN = 12
H = 128
NCORES = 8
SH = N // NCORES          # 6250 nodes per shard
CH = 6272                 # padded chunk rows (6250 + 22 zero pad) = 49*128
BLK = CH // 128           # 49 blocks per shard
LASTB = SH - (BLK - 1) * 128  # 106 valid rows in last block
HALF = 4 * CH             # 25088 rows per gather half (int16-safe)
ZROW = 0                  # pad gather row: any finite row (S row is zero)
PADCREL = 300.0           # colrel value guaranteed not to match 0..127

bf16 = ml_dtypes.bfloat16

LAST_RESULTS = None


def _prep_edges(edge_index):
    """Partition/sort edges (NO self-loops); build per-core gather-index and
    colrel streams with slot sizes (TL) uniform across cores."""
    row = np.asarray(edge_index[0], np.int64)
    col = np.asarray(edge_index[1], np.int64)
    deg = (np.bincount(row, minlength=N) + 1).astype(np.float32)  # + self loop

    s_shard = row // SH
    grow = s_shard * CH + (row % SH)       # row index in gathered table
    half = (grow >= HALF).astype(np.int64)
    gloc = np.where(half == 0, grow, grow - HALF)
    tcore = col // SH
    tcol = col % SH
    blk = tcol // 128
    crel = (tcol % 128).astype(np.int64)

    key = ((tcore * BLK) + blk) * 2 + half
    order = np.lexsort((gloc, key))
    key_s = key[order]
    gloc_s = gloc[order]
    crel_s = crel[order]

    nslots = NCORES * BLK * 2
    cnt = np.bincount(key_s, minlength=nslots).reshape(NCORES, BLK, 2)
    starts = np.zeros(nslots + 1, np.int64)
    np.cumsum(cnt.reshape(-1), out=starts[1:])

    TL = np.maximum(1, ((cnt + 127) // 128).max(axis=0))  # [BLK, 2]
    off = np.zeros((BLK, 2), np.int64)
    tot = [0, 0]
    for h in (0, 1):
        for b in range(BLK):
            off[b, h] = tot[h]
            tot[h] += TL[b, h]

    gidx = []   # per core: ((gi_lo, cr_lo), (gi_hi, cr_hi))
    for k in range(NCORES):
        per_half = []
        for h in (0, 1):
            gparts, cparts = [], []
            for b in range(BLK):
                s = starts[(k * BLK + b) * 2 + h]
                e = starts[(k * BLK + b) * 2 + h + 1]
                n = int(e - s)
                m = int(TL[b, h]) * 128
                gseg = np.full(m, ZROW, np.int64)
                cseg = np.full(m, PADCREL, np.float64)
                gseg[:n] = gloc_s[s:e]
                cseg[:n] = crel_s[s:e]
                # wrapped idx layout: pos i -> partition i%16, col i//16
                gparts.append(gseg.reshape(m // 16, 16).T.astype(np.int16))
                # colrel layout: pos i -> partition i%128, col i//128
                cparts.append(cseg.reshape(m // 128, 128).T.astype(bf16))
            g = np.hstack(gparts)   # [16, 8*TOT]; replicated to 128 on device
            per_half.append((np.ascontiguousarray(g), np.hstack(cparts).copy()))
        gidx.append(per_half)
    return deg, TL, off, gidx


def _build_program(TL, off):
    import concourse.bacc as bacc
    import concourse.mybir as mybir
    import concourse.tile as tile

    dt = mybir.dt
    nc = bacc.Bacc("TRN2", target_bir_lowering=False, debug=False,
                   num_devices=NCORES)

    TOT = [int(off[-1, 0] + TL[-1, 0]), int(off[-1, 1] + TL[-1, 1])]
    TLMAX = int(TL.max())

    # ---- inputs ----
    t1pad = nc.dram_tensor("t1pad", [NCORES * CH, H], dt.bfloat16,
                           kind="ExternalInput")
    t1own = nc.dram_tensor("t1own", [CH, 16], dt.bfloat16, kind="ExternalInput")
    W1aug = nc.dram_tensor("W1aug", [16, H], dt.bfloat16, kind="ExternalInput")
    W2T = nc.dram_tensor("W2T", [H, H], dt.bfloat16, kind="ExternalInput")
    b2r = nc.dram_tensor("b2r", [1, H], dt.bfloat16, kind="ExternalInput")
    W3T = nc.dram_tensor("W3T", [H, 16], dt.bfloat16, kind="ExternalInput")
    b3r = nc.dram_tensor("b3r", [1, 16], dt.bfloat16, kind="ExternalInput")
    sqdeg = nc.dram_tensor("sqdeg", [1, CH], dt.bfloat16, kind="ExternalInput")
    dinvc = nc.dram_tensor("dinvc", [128, BLK], dt.float32, kind="ExternalInput")
    dinv2c = nc.dram_tensor("dinv2c", [128, BLK], dt.float32, kind="ExternalInput")
    id128 = nc.dram_tensor("id128", [128, H], dt.bfloat16, kind="ExternalInput")
    id16 = nc.dram_tensor("id16", [16, 16], dt.bfloat16, kind="ExternalInput")
    io3 = nc.dram_tensor("io3", [128, 128 * TLMAX], dt.bfloat16,
                         kind="ExternalInput")
    Rbf = nc.dram_tensor("Rbf", [128, C * C], dt.bfloat16, kind="ExternalInput")
    gi_lo = nc.dram_tensor("gi_lo", [16, 8 * TOT[0]], dt.int16,
                           kind="ExternalInput")
    gi_hi = nc.dram_tensor("gi_hi", [16, 8 * TOT[1]], dt.int16,
                           kind="ExternalInput")
    cr_lo = nc.dram_tensor("cr_lo", [128, TOT[0]], dt.bfloat16,
                           kind="ExternalInput")
    cr_hi = nc.dram_tensor("cr_hi", [128, TOT[1]], dt.bfloat16,
                           kind="ExternalInput")
    out = nc.dram_tensor("out", [SH, C], dt.float32, kind="ExternalOutput")

    gin2 = nc.dram_tensor("gin2", [CH, H], dt.bfloat16)
    gout2 = nc.dram_tensor("gout2", [NCORES * CH, H], dt.bfloat16,
                           addr_space="Shared")
    gin3 = nc.dram_tensor("gin3", [CH, 16], dt.bfloat16)
    gout3 = nc.dram_tensor("gout3", [NCORES * CH, 16], dt.bfloat16,
                           addr_space="Shared")
    g3pad = nc.dram_tensor("g3pad", [NCORES * CH, H], dt.bfloat16)

    with tile.TileContext(nc) as tc:
        with (
            tc.tile_pool(name="const", bufs=1) as cpool,
            tc.tile_pool(name="idx", bufs=1) as ipool,
            tc.tile_pool(name="msg", bufs=6) as mpool,
            tc.tile_pool(name="sbl", bufs=6) as spool,
            tc.tile_pool(name="hsm", bufs=4) as hpool,
            tc.tile_pool(name="psum", bufs=3, space="PSUM") as pp,
            tc.tile_pool(name="psumt", bufs=4, space="PSUM") as ppt,
        ):
            # ---- constants ----
            def load(pool, shape, dtype, src, tag):
                t = pool.tile(shape, dtype, tag=tag)
                nc.sync.dma_start(out=t[:], in_=src)
                return t

            # load idx rows into partitions 0:16, then replicate x8 on DVE
            # (dma_gather needs the wrapped idx physically in all 128 parts)
            gil_t = ipool.tile([128, 8 * TOT[0]], dt.int16, tag="gil")
            nc.sync.dma_start(out=gil_t[0:16, :], in_=gi_lo[:])
            gih_t = ipool.tile([128, 8 * TOT[1]], dt.int16, tag="gih")
            nc.sync.dma_start(out=gih_t[0:16, :], in_=gi_hi[:])
            for t in (gil_t, gih_t):
                # DVE writes need 32-partition alignment; first doubling via
                # a small SBUF->SBUF DMA instead
                nc.sync.dma_start(out=t[16:32, :], in_=t[0:16, :])
                for lo in (32, 64):
                    nc.vector.tensor_copy(t[lo:2 * lo, :], t[0:lo, :])
            crl_t = load(ipool, [128, TOT[0]], dt.bfloat16, cr_lo[:], "crl")
            crh_t = load(ipool, [128, TOT[1]], dt.bfloat16, cr_hi[:], "crh")
            w1_t = load(cpool, [16, H], dt.bfloat16, W1aug[:], "w1")
            w2_t = load(cpool, [H, H], dt.bfloat16, W2T[:], "w2")
            b2_t = load(cpool, [1, H], dt.bfloat16, b2r[:], "b2")
            w3_t = load(cpool, [H, 16], dt.bfloat16, W3T[:], "w3")
            b3_t = load(cpool, [1, 16], dt.bfloat16, b3r[:], "b3")
            sq_t = load(cpool, [1, CH], dt.bfloat16, sqdeg[:], "sq")
            dinv_t = load(cpool, [128, BLK], dt.float32, dinvc[:], "dinv")
            dinv2_t = load(cpool, [128, BLK], dt.float32, dinv2c[:], "dinv2")
            id128_t = load(cpool, [128, H], dt.bfloat16, id128[:], "id128")
            id16_t = load(cpool, [16, 16], dt.bfloat16, id16[:], "id16")
            io3_t = load(cpool, [128, 128, TLMAX], dt.bfloat16,
                         io3[:].rearrange("p (c j) -> p c j", c=128), "io3")
            r_t = load(cpool, [128, C * C], dt.bfloat16, Rbf[:], "r")
            t1o_t = load(cpool, [128, BLK, 16], dt.bfloat16,
                         t1own[:].rearrange("(b p) f -> p b f", p=128), "t1o")
            # table-block cache (own shard) for next layer's self-loop matmul
            gc2 = cpool.tile([128, BLK * 128], dt.bfloat16, tag="gc2")
            gc3 = cpool.tile([128, BLK * 16], dt.bfloat16, tag="gc3")

            halves2 = ((gil_t, crl_t), (gih_t, crh_t))

            def agg_block(b, src_lo, src_hi, width, self_lhsT, tag):
                """Transposed gather+scatter for node block b.
                Returns PSUM tile [width, 128] = sum_e msg[e,:width]^T S[e,:]
                (+ self-loop term)."""
                acc = pp.tile([width, 128], dt.float32, tag="agg")
                nc.tensor.matmul(acc[:, :], self_lhsT, id128_t[:, :],
                                 start=True, stop=False)
                srcs = (src_lo, src_hi)
                for h in (0, 1):
                    gi_t, cr_t = halves2[h]
                    tl = int(TL[b, h])
                    o = int(off[b, h])
                    msg = mpool.tile([128, TLMAX, H], dt.bfloat16, tag="msg")
                    nc.gpsimd.dma_gather(
                        out_ap=msg[:, 0:tl, :], in_ap=srcs[h],
                        idxs_ap=gi_t[:, o * 8:(o + tl) * 8],
                        num_idxs=tl * 128, num_idxs_reg=tl * 128, elem_size=H,
                        single_packet=False,
                    )
                    S = spool.tile([128, 128, TLMAX], dt.bfloat16, tag="sb")
                    nc.vector.tensor_tensor(
                        out=S[:, :, 0:tl],
                        in0=cr_t[:, o:o + tl].unsqueeze(1)
                            .broadcast_to([128, 128, tl]),
                        in1=io3_t[:, :, 0:tl],
                        op=mybir.AluOpType.is_equal,
                    )
                    for j in range(tl):
                        last = (h == 1 and j == int(TL[b, 1]) - 1)
                        nc.tensor.matmul(acc[:, :], msg[:, j, 0:width],
                                         S[:, :, j], start=False, stop=last)
                return acc

            # ---------------- Layer 1: agg(t1) -> transform -> table2 -------
            for b in range(BLK):
                bs, be = b * 128, (b + 1) * 128
                acc1 = agg_block(b, t1pad[0:HALF, :], t1pad[HALF:2 * HALF, :],
                                 16, t1o_t[:, b, :], "agg")
                htbA = hpool.tile([16, 128], dt.bfloat16, tag="htbA")
                nc.scalar.activation(htbA[:, :], acc1[:, :],
                                     mybir.ActivationFunctionType.Copy)
                acc2T = ppt.tile([H, 128], dt.float32, tag="tf")
                nc.tensor.matmul(acc2T[:, :], w1_t[:, :], htbA[:, :],
                                 start=True, stop=True)
                h1T = hpool.tile([H, 128], dt.bfloat16, tag="hT")
                nc.scalar.activation(h1T[:, :], acc2T[:, :],
                                     mybir.ActivationFunctionType.Relu)
                acc3 = ppt.tile([128, H], dt.float32, tag="tf")
                nc.tensor.matmul(acc3[:, :], h1T[:, :], w2_t[:, :],
                                 start=True, stop=False)
                nc.tensor.matmul(acc3[:, :], sq_t[0:1, bs:be], b2_t[:, :],
                                 start=False, stop=True)
                nc.scalar.activation(gc2[:, bs:be], acc3[:, :],
                                     mybir.ActivationFunctionType.Copy,
                                     scale=dinv2_t[:, b:b + 1])
                nc.sync.dma_start(out=gin2[bs:be, :], in_=gc2[:, bs:be])

            nc.gpsimd.collective_compute(
                "AllGather", mybir.AluOpType.bypass,
                replica_groups=[list(range(NCORES))],
                ins=[gin2[:, :]], outs=[gout2[:, :]],
            )

            # ---------------- Layer 2: agg(t2) -> transform -> table3 -------
            for b in range(BLK):
                bs, be = b * 128, (b + 1) * 128
                acc4 = agg_block(b, gout2[0:HALF, :], gout2[HALF:2 * HALF, :],
                                 H, gc2[:, bs:be], "agg")
                h2T = hpool.tile([H, 128], dt.bfloat16, tag="hT")
                nc.scalar.activation(h2T[:, :], acc4[:, :],
                                     mybir.ActivationFunctionType.Relu)
                acc5 = ppt.tile([128, 16], dt.float32, tag="tf")
                nc.tensor.matmul(acc5[:, :], h2T[:, :], w3_t[:, :],
                                 start=True, stop=False)
                nc.tensor.matmul(acc5[:, :], sq_t[0:1, bs:be], b3_t[:, :],
                                 start=False, stop=True)
                nc.scalar.activation(gc3[:, b * 16:(b + 1) * 16], acc5[:, :],
                                     mybir.ActivationFunctionType.Copy,
                                     scale=dinv2_t[:, b:b + 1])
                nc.sync.dma_start(out=gin3[bs:be, :],
                                  in_=gc3[:, b * 16:(b + 1) * 16])

            nc.gpsimd.collective_compute(
                "AllGather", mybir.AluOpType.bypass,
                replica_groups=[list(range(NCORES))],
                ins=[gin3[:, :]], outs=[gout3[:, :]],
            )
            # pad-expand compact 16-wide table into 256B-stride gather rows
            # (split per gather-half so half-0 gathers start during half-1)
            nc.sync.dma_start(out=g3pad[0:HALF, 0:16], in_=gout3[0:HALF, :])
            nc.sync.dma_start(out=g3pad[HALF:2 * HALF, 0:16],
                              in_=gout3[HALF:2 * HALF, :])

            # ---------------- Layer 3: agg(t3) -> sigmoid -> hierarchy max --
            for b in range(BLK):
                bs = b * 128
                acc6 = agg_block(b, g3pad[0:HALF, :], g3pad[HALF:2 * HALF, :],
                                 16, gc3[:, b * 16:(b + 1) * 16], "agg")
                sb6 = hpool.tile([16, 128], dt.bfloat16, tag="htbA")
                nc.scalar.activation(sb6[:, :], acc6[:, :],
                                     mybir.ActivationFunctionType.Copy)
                acc7 = ppt.tile([128, 16], dt.float32, tag="tf")
                nc.tensor.matmul(acc7[:, :], sb6[:, :], id16_t[:, :],
                                 start=True, stop=True)
                s7 = hpool.tile([128, 16], dt.bfloat16, tag="s7")
                nc.scalar.activation(s7[:, :], acc7[:, :],
                                     mybir.ActivationFunctionType.Sigmoid,
                                     scale=dinv_t[:, b:b + 1])
                tmp = hpool.tile([128, C, C], dt.bfloat16, tag="tmp")
                nc.vector.tensor_tensor(
                    out=tmp[:, :, :],
                    in0=s7[:, 0:C].unsqueeze(1).broadcast_to([128, C, C]),
                    in1=r_t[:, :].rearrange("p (a b) -> p a b", a=C),
                    op=mybir.AluOpType.mult,
                )
                o13 = hpool.tile([128, C], dt.float32, tag="o13")
                nc.vector.tensor_reduce(o13[:, :], tmp[:, :, :],
                                        axis=mybir.AxisListType.X,
                                        op=mybir.AluOpType.max)
                rows = 128 if b < BLK - 1 else LASTB
                nc.sync.dma_start(out=out[bs:bs + rows, :],
                                  in_=o13[0:rows, :])

    nc.compile()
    return nc


def _host_tensors(x, R, W1, b1, W2, b2, W3, b3, deg, TL):
    """Replicated + per-core host-prepared input tensors."""
    TLMAX = int(TL.max())
    dinv = 1.0 / np.sqrt(deg)

    # layer-1 table rows: dinv_r * [x_r, 1, 0...] (bf16, CH-strided chunks)
    t1pad = np.zeros([NCORES * CH, H], bf16)
    t1c = np.zeros([NCORES, CH, 16], bf16)
    for k in range(NCORES):
        sl = slice(k * SH, (k + 1) * SH)
        rows = np.zeros([CH, 16], np.float32)
        rows[:SH, :DIN] = x[sl] * dinv[sl, None]
        rows[:SH, DIN] = dinv[sl]
        t1c[k] = rows.astype(bf16)
        t1pad[k * CH:(k + 1) * CH, :16] = rows.astype(bf16)

    W1a = np.zeros([16, H], np.float32)
    W1a[:DIN] = np.asarray(W1, np.float32).T
    W1a[DIN] = np.asarray(b1, np.float32)
    W3Tp = np.zeros([H, 16], np.float32)
    W3Tp[:, :C] = np.asarray(W3, np.float32).T
    b3p = np.zeros([1, 16], np.float32)
    b3p[0, :C] = np.asarray(b3, np.float32)

    io3 = np.tile(np.arange(128, dtype=np.float32)[None, :, None],
                  (128, 1, TLMAX)).reshape(128, 128 * TLMAX)

    common = {
        "t1pad": t1pad,
        "W1aug": W1a.astype(bf16),
        "W2T": np.ascontiguousarray(np.asarray(W2, np.float32).T).astype(bf16),
        "b2r": np.asarray(b2, np.float32)[None, :].astype(bf16),
        "W3T": W3Tp.astype(bf16),
        "b3r": b3p.astype(bf16),
        "id128": np.eye(128, dtype=np.float32).astype(bf16),
        "id16": np.eye(16, dtype=np.float32).astype(bf16),
        "io3": io3.astype(bf16),
        "Rbf": np.tile(np.asarray(R, np.float32).reshape(1, C * C),
                       (128, 1)).astype(bf16),
    }

    per_core = []
    for k in range(NCORES):
        sl = slice(k * SH, (k + 1) * SH)
        sq = np.zeros([1, CH], np.float32)
        sq[0, :SH] = np.sqrt(deg[sl])
        dv = np.ones([CH], np.float32)
        dv[:SH] = dinv[sl]
        dvc = np.ascontiguousarray(dv.reshape(BLK, 128).T)
        dv2 = np.zeros([CH], np.float32)
        dv2[:SH] = dinv[sl] ** 2
        dv2c = np.ascontiguousarray(dv2.reshape(BLK, 128).T)
        per_core.append({
            "t1own": np.ascontiguousarray(t1c[k]),
            "sqdeg": sq.astype(bf16),
            "dinvc": dvc,
            "dinv2c": dv2c,
        })
    return common, per_core


def kernel(x, edge_index, R, W1, b1, W2, b2, W3, b3, **_):
    global LAST_RESULTS
    import concourse.mybir  # noqa: F401
    from concourse.bass_utils import run_bass_kernel_spmd

    x = np.asarray(x, np.float32)
    edge_index = np.asarray(edge_index, np.int32)
    deg, TL, off, gidx = _prep_edges(edge_index)

    nc = _build_program(TL, off)
    common, per_core = _host_tensors(x, R, W1, b1, W2, b2, W3, b3, deg, TL)

    in_maps = []
    for k in range(NCORES):
        (g_lo, c_lo), (g_hi, c_hi) = gidx[k]
        m = dict(common)
        m.update(per_core[k])
        m.update({"gi_lo": g_lo, "gi_hi": g_hi, "cr_lo": c_lo, "cr_hi": c_hi})
        in_maps.append(m)

    trace = os.environ.get("GNN_TRACE") == "1"
    res = run_bass_kernel_spmd(nc, in_maps, core_ids=list(range(NCORES)),
                               trace=trace)
    LAST_RESULTS = res

    reps = int(os.environ.get("GNN_BENCH", "0"))
    if reps > 0:
        _bench(nc, in_maps, reps)
    return np.concatenate([res.results[k]["out"] for k in range(NCORES)], axis=0)


BENCH_TIMES = None
BENCH_PIPELINED_NS = None


def _bench(nc, in_maps, reps):
    """Time repeated executions of the already-built program through a single
    jit instance (NEFF compile amortized away; inputs device_put once)."""
    global BENCH_TIMES
    import time
    import jax
    from jax.sharding import Mesh, PartitionSpec, NamedSharding
    from jax.experimental.shard_map import shard_map
    import concourse.mybir as mybir
    from concourse.bass2jax import (_bass_exec_p, partition_id_tensor,
                                    install_neuronx_cc_hook)

    install_neuronx_cc_hook()
    in_names, out_names, out_avals, zero_outs = [], [], [], []
    pname = nc.partition_id_tensor.name if nc.partition_id_tensor else None
    for alloc in nc.m.functions[0].allocations:
        if not isinstance(alloc, mybir.MemoryLocationSet):
            continue
        name = alloc.memorylocations[0].name
        if alloc.kind == "ExternalInput":
            if name != pname:
                in_names.append(name)
        elif alloc.kind == "ExternalOutput":
            out_names.append(name)
            shape = tuple(alloc.tensor_shape)
            dtype = mybir.dt.np(alloc.dtype)
            out_avals.append(jax.core.ShapedArray(shape, dtype))
            zero_outs.append(np.zeros(shape, dtype))
    n_params = len(in_names)
    all_names = in_names + out_names + ([pname] if pname else [])

    def _body(*args):
        ops = list(args)
        if pname:
            ops.append(partition_id_tensor())
        return tuple(_bass_exec_p.bind(
            *ops, out_avals=tuple(out_avals), in_names=tuple(all_names),
            out_names=tuple(out_names), lowering_input_output_aliases=(),
            sim_require_finite=True, sim_require_nnan=True, nc=nc))

    devices = jax.devices()[:NCORES]
    mesh = Mesh(np.asarray(devices), ("core",))
    nouts = len(out_names)
    sharded = jax.jit(
        shard_map(_body, mesh=mesh,
                  in_specs=(PartitionSpec("core"),) * (n_params + nouts),
                  out_specs=(PartitionSpec("core"),) * nouts, check_rep=False),
        donate_argnums=tuple(range(n_params, n_params + nouts)),
        keep_unused=True)
    sh = NamedSharding(mesh, PartitionSpec("core"))
    dev_in = [jax.device_put(
        np.concatenate([np.asarray(in_maps[c][nm]) for c in range(NCORES)], axis=0), sh)
        for nm in in_names]
    times = []
    for i in range(reps + 1):
        zs = [jax.device_put(
            np.zeros((NCORES * z.shape[0], *z.shape[1:]), z.dtype), sh)
            for z in zero_outs]
        t0 = time.perf_counter()
        outs = sharded(*dev_in, *zs)
        jax.block_until_ready(outs)
        times.append(time.perf_counter() - t0)
    BENCH_TIMES = times
    print("bench wall times (s):", " ".join(f"{t:.4f}" for t in times))
    print(f"bench min/median after warmup: {min(times[1:]):.4f} / "
          f"{sorted(times[1:])[len(times[1:]) // 2]:.4f}")

    # pipelined async dispatch: amortizes per-call RPC overhead; min of 3
    # rounds (wall noise through axon is ~+-1.3 ms)
    NPIPE = 20
    best = None
    for rnd in range(3):
        zss = [[jax.device_put(
            np.zeros((NCORES * z.shape[0], *z.shape[1:]), z.dtype), sh)
            for z in zero_outs] for _ in range(NPIPE)]
        t0 = time.perf_counter()
        outs = None
        for i in range(NPIPE):
            outs = sharded(*dev_in, *zss[i])
        jax.block_until_ready(outs)
        tp = (time.perf_counter() - t0) / NPIPE
        print(f"bench pipelined round {rnd}: {tp * 1e3:.3f} ms")
        best = tp if best is None else min(best, tp)
    global BENCH_PIPELINED_NS
    BENCH_PIPELINED_NS = int(best * 1e9)
    print(f"bench pipelined per-exec: {best * 1e3:.3f} ms "
          f"({best * 1e9:.0f} ns upper bound)")
